# revision 16
# baseline (speedup 1.0000x reference)
"""Trainium2 Bass kernel for nn_Decoder (4-layer post-LN transformer decoder).

Sharding: 8 cores = 4 batches x 2 sequence-halves (256 tokens each).
Activations on-chip live transposed [E, tok]. Per layer a pair-wise
AllGather shares the hidden state so self-attn K/V sees the full 512-token
sequence. fc_out is vocab-sharded 8 ways after a full AllGather.
Compute: bf16 matmul operands, f32 accumulation / softmax / LN.
"""

import numpy as np
import ml_dtypes
from contextlib import ExitStack

import concourse.bass as bass
import concourse.tile as tile
from concourse import mybir
from concourse.bass_utils import run_bass_kernel_spmd
from concourse.masks import make_identity

BF16 = ml_dtypes.bfloat16

# Problem dims (hardcoded)
N, T, S, E, H, V, L, FF, NL = 4, 512, 512, 1024, 16, 32000, 1024, 4096, 4
HD = E // H          # 64
NC_CORES = 8
TOK = 256            # tokens per core
VS = 4096            # vocab shard (padded 32768 / 8)
VPAD = VS * NC_CORES
ET = E // 128        # 8
FT = FF // 128       # 32
KT = S // 128        # 4
P = 128
NTOK_ALL = NC_CORES * TOK  # 2048

DT = mybir.dt
AX = mybir.AxisListType
ALU = mybir.AluOpType
AF = mybir.ActivationFunctionType

PAIRS = [[0, 1], [2, 3], [4, 5], [6, 7]]
ALL8 = [list(range(NC_CORES))]

_CACHE = {}
LAST_RESULTS = None  # BassKernelResults of most recent run (for test.py)


# ---------------------------------------------------------------- builder ---

def _inp(nc, d, name, shape, dtype):
    d[name] = nc.declare_dram_parameter(name, list(shape), dtype, isOutput=False)
    return d[name]


def build_nc():
    nc = bass.Bass(num_devices=NC_CORES)
    f32, bf, i32 = DT.float32, DT.bfloat16, DT.int32
    d = {}
    _inp(nc, d, "idx", [TOK], i32)
    _inp(nc, d, "pos_t", [E, TOK], f32)
    _inp(nc, d, "enc_t", [E, S], bf)
    _inp(nc, d, "tmask", [TOK, T], f32)
    _inp(nc, d, "smask", [1, S], f32)
    _inp(nc, d, "wemb", [V, E], bf)
    _inp(nc, d, "fcw", [FT, P, E], bf)
    _inp(nc, d, "fcb", [VS], f32)
    for l in range(NL):
        for w in ("swq", "swk", "swv", "cwq", "cwk", "cwv"):
            _inp(nc, d, f"l{l}_{w}", [P, P], bf)
        _inp(nc, d, f"l{l}_swo", [ET, P, E], bf)
        _inp(nc, d, f"l{l}_cwo", [ET, P, E], bf)
        _inp(nc, d, f"l{l}_w1", [FT, P, E], bf)
        _inp(nc, d, f"l{l}_w2", [ET, P, FF], bf)
        _inp(nc, d, f"l{l}_gb", [E, 6], f32)   # columns: g0 b0 g1 b1 g2 b2
        _inp(nc, d, f"l{l}_bov", [E, 3], f32)  # columns: bo_self bo_cross b2
        _inp(nc, d, f"l{l}_b1", [FF], f32)
    out_d = nc.declare_dram_parameter("out", [VS, NTOK_ALL], f32, isOutput=True)

    with tile.TileContext(nc) as tc:
        with ExitStack() as ctx:
            _build_program(ctx, tc, d, out_d)
    _spill_excess_waits(nc)
    return nc


def _spill_excess_waits(nc):
    """This walrus build caps sync waits at 1 per instruction (2 for
    EventSemaphore). Move extra waits onto standalone EventSemaphore
    instructions inserted just before the offender on the same engine —
    serial waits on the same queue are semantically identical to a
    combined wait list."""
    n_new = [0]

    def mk_ev(engine, waits, debug):
        n_new[0] += 1
        ev = mybir.InstEventSemaphore(
            name=f"EVW-{n_new[0]}", engine=engine, ins=[], outs=[],
            sync_info=mybir.SyncInfo(on_wait=list(waits), on_update=[]),
            debug=debug)
        return ev

    for fn in nc.m.functions:
        for blk in fn.blocks:
            out = []
            changed = False
            for inst in blk.instructions:
                si = inst.sync_info
                cap = 2 if isinstance(inst, mybir.InstEventSemaphore) else 1
                if si is not None and len(si.on_wait) > cap:
                    waits = list(si.on_wait)
                    extra, keep = waits[:-cap], waits[-cap:]
                    for i in range(0, len(extra), 2):
                        out.append(mk_ev(inst.engine, extra[i:i + 2], inst.debug))
                    inst.sync_info = mybir.SyncInfo(
                        on_wait=keep, on_update=list(si.on_update))
                    changed = True
                out.append(inst)
            if changed:
                blk.instructions = out


def _build_program(ctx, tc, d, out_d):
    nc = tc.nc
    f32, bf = DT.float32, DT.bfloat16

    pp = ctx.enter_context(tc.tile_pool(name="pp", bufs=1, space="PSUM"))
    sb = ctx.enter_context(tc.tile_pool(name="sb", bufs=1))
    dr = ctx.enter_context(tc.tile_pool(name="dr", bufs=1, space="DRAM"))

    def psum(shape, tag, bufs, dtype=f32):
        return pp.tile(shape, dtype, tag=tag, bufs=bufs, name=tag)

    def st(shape, dtype, tag, bufs):
        return sb.tile(shape, dtype, tag=tag, bufs=bufs, name=tag)

    cp_i = [0]

    def cpy(out, in_):
        # alternate bulk PSUM->SBUF copies between ACT and DVE
        cp_i[0] += 1
        if cp_i[0] % 2:
            nc.scalar.copy(out, in_)
        else:
            nc.vector.tensor_copy(out, in_)

    # ---- constants -------------------------------------------------------
    ident_f = st([P, P], f32, "ident_f", 1)
    make_identity(nc, ident_f[:, :])
    ident_b = st([P, P], bf, "ident_b", 1)
    nc.vector.tensor_copy(ident_b, ident_f)
    ones_f = st([P, 1], f32, "ones_f", 1)
    nc.vector.memset(ones_f, 1.0)
    ones_b = st([P, 1], bf, "ones_b", 1)
    nc.vector.memset(ones_b, 1.0)
    ones1_f = st([1, P], f32, "ones1_f", 1)
    nc.vector.memset(ones1_f, 1.0)
    ones1_b = st([1, P], bf, "ones1_b", 1)
    nc.vector.memset(ones1_b, 1.0)
    eps_t = st([1, 1], f32, "eps_t", 1)
    nc.vector.memset(eps_t, 1e-5)

    # masks
    madd_self = []
    for qt in range(2):
        tmt = st([P, T], f32, "tm", 2)
        nc.sync.dma_start(out=tmt, in_=d["tmask"][qt * P:(qt + 1) * P, :])
        ms = st([P, T], bf, f"madd_self{qt}", 1)
        nc.vector.tensor_scalar(out=ms, in0=tmt, scalar1=1e20, scalar2=1e20,
                                op0=ALU.mult, op1=ALU.subtract)
        madd_self.append(ms)
    smr = st([1, S], f32, "smr", 1)
    nc.sync.dma_start(out=smr, in_=d["smask"][:, :])
    madd_cross = st([1, S], bf, "madd_cross", 1)
    nc.vector.tensor_scalar(out=madd_cross, in0=smr, scalar1=1e20, scalar2=1e20,
                            op0=ALU.mult, op1=ALU.subtract)

    # encoder (transposed) resident bf16
    enc_sb = []
    for e in range(ET):
        t = st([P, S], bf, "enc_sb", ET)
        nc.sync.dma_start(out=t, in_=d["enc_t"][e * P:(e + 1) * P, :])
        enc_sb.append(t)

    # ---- embedding -------------------------------------------------------
    resid = []
    h_bf = []
    for e in range(ET):
        resid.append(st([P, TOK], f32, "resid", 9))
        h_bf.append(st([P, TOK], bf, "hbf", 10))
    pos_sb = []
    for e in range(ET):
        ps = st([P, TOK], f32, "xc", 10)
        nc.sync.dma_start(out=ps, in_=d["pos_t"][e * P:(e + 1) * P, :])
        pos_sb.append(ps)
    for tb in range(2):
        it = st([P, 1], DT.int32, "idx", 2)
        nc.sync.dma_start(out=it, in_=d["idx"][tb * P:(tb + 1) * P, None])
        g = st([P, E], bf, "gath", 2)
        nc.gpsimd.indirect_dma_start(
            out=g, out_offset=None, in_=d["wemb"][:, :],
            in_offset=bass.IndirectOffsetOnAxis(ap=it[:, :1], axis=0))
        for e in range(ET):
            tp = psum([P, P], "medb", 2, dtype=bf)
            nc.tensor.transpose(out=tp, in_=g[:, e * P:(e + 1) * P], identity=ident_b)
            nc.vector.tensor_tensor(out=resid[e][:, tb * P:(tb + 1) * P],
                                    in0=tp, in1=pos_sb[e][:, tb * P:(tb + 1) * P],
                                    op=ALU.add)
    for e in range(ET):
        nc.scalar.copy(h_bf[e], resid[e])

    # ---- helpers ---------------------------------------------------------
    def load_wcol(name, tag, bufs):
        # [dim] f32 -> SBUF [128, dim/128]; [dim, c] f32 -> SBUF [128, dim/128, c]
        src = d[name]
        if len(src.shape) == 1:
            t = st([P, src.shape[0] // P], f32, tag, bufs)
            nc.sync.dma_start(out=t, in_=src.rearrange("(a p) -> p a", p=P))
        else:
            t = st([P, src.shape[0] // P, src.shape[1]], f32, tag, bufs)
            nc.sync.dma_start(out=t, in_=src.rearrange("(a p) c -> p a c", p=P))
        return t

    def layer_norm(r2, gcol, bcol, lname):
        """r2: list of 8 [P,TOK] f32 SBUF tiles. gcol/bcol: [P,1] APs per etile.
        Returns (resid_new, ybf) lists."""
        sums = psum([1, TOK], "stat", 1)
        for e in range(ET):
            nc.tensor.matmul(out=sums, lhsT=ones_f, rhs=r2[e],
                             start=(e == 0), stop=(e == ET - 1))
        mu_row = st([1, TOK], f32, "mu_row", 2)
        nc.scalar.mul(mu_row, sums, 1.0 / E)
        Mu = psum([P, TOK], "med", 2)
        nc.tensor.matmul(out=Mu, lhsT=ones1_f, rhs=mu_row, start=True, stop=True)
        xc = []
        ssq = psum([1, TOK], "stat", 1)
        for e in range(ET):
            x = st([P, TOK], f32, "xc", 10)
            nc.vector.tensor_tensor(out=x, in0=r2[e], in1=Mu, op=ALU.subtract)
            xc.append(x)
            sq = st([P, TOK], bf, "sq", 4)
            nc.vector.tensor_tensor(out=sq, in0=x, in1=x, op=ALU.mult)
            nc.tensor.matmul(out=ssq, lhsT=ones_b, rhs=sq,
                             start=(e == 0), stop=(e == ET - 1))
        std_row = st([1, TOK], f32, "std_row", 2)
        nc.scalar.activation(out=std_row, in_=ssq, func=AF.Sqrt,
                             bias=eps_t[:, :], scale=1.0 / E)
        rstd_row = st([1, TOK], f32, "rstd_row", 2)
        nc.vector.reciprocal(rstd_row, std_row)
        Rstd = psum([P, TOK], "med", 2)
        nc.tensor.matmul(out=Rstd, lhsT=ones1_f, rhs=rstd_row, start=True, stop=True)
        rn, yb = [], []
        for e in range(ET):
            xn = st([P, TOK], f32, "xn", 3)
            nc.vector.tensor_tensor(out=xn, in0=xc[e], in1=Rstd, op=ALU.mult)
            r = st([P, TOK], f32, "resid", 9)
            nc.vector.tensor_scalar(out=r, in0=xn, scalar1=gcol[e], scalar2=bcol[e],
                                    op0=ALU.mult, op1=ALU.add)
            y = st([P, TOK], bf, "hbf", 10)
            nc.scalar.copy(y, r)
            rn.append(r)
            yb.append(y)
        return rn, yb

    def attention(l, q_src, kT, v_sb, nkt, self_mask, wo_name, res_in, bocol):
        """q_src: 8 bf16 [P,TOK] tiles. kT: 8 [P, nkt*128] bf16. v_sb: nkt
        [P,E] bf16. Returns r2 list (resid + attn out + bo), pre-LN."""
        wq = st([P, P], bf, "wbd", 6)
        nc.sync.dma_start(out=wq, in_=d[f"l{l}_{'swq' if self_mask else 'cwq'}"][:, :])
        qT = []
        for e in range(ET):
            qp = psum([P, TOK], "big", 3)
            nc.tensor.matmul(out=qp, lhsT=wq, rhs=q_src[e], start=True, stop=True)
            q = st([P, TOK], bf, "qT", ET)
            cpy(q, qp)
            qT.append(q)
        nk = nkt * P
        probs = {}
        for h in range(H):
            ep_pair, r0 = h // 2, (h % 2) * 64
            for qt in range(2):
                ep = psum([P, nk], "big", 3)
                nc.tensor.matmul(
                    out=ep,
                    lhsT=qT[ep_pair][r0:r0 + 64, qt * P:(qt + 1) * P],
                    rhs=kT[ep_pair][r0:r0 + 64, :],
                    start=True, stop=False)
                if self_mask:
                    nc.tensor.matmul(out=ep, lhsT=ident_b, rhs=madd_self[qt],
                                     start=False, stop=True)
                else:
                    nc.tensor.matmul(out=ep, lhsT=ones1_b, rhs=madd_cross,
                                     start=False, stop=True)
                mx = st([P, 1], f32, "mx", 6)
                nc.vector.reduce_max(mx, ep, axis=AX.X, negate=True)
                pr = st([P, nk], bf, "probs", 6)
                sm = st([P, 1], f32, "sm", 6)
                nc.scalar.activation(out=pr, in_=ep, func=AF.Exp,
                                     bias=mx, scale=1.0, accum_out=sm)
                rc = st([P, 1], f32, "rc", 6)
                nc.vector.reciprocal(rc, sm)
                nc.vector.tensor_scalar_mul(out=pr, in0=pr, scalar1=rc)
                probs[(h, qt)] = pr
        # transpose probs -> [k, q] then av
        avT = [st([P, TOK], bf, "avT", ET) for _ in range(ET)]
        for h in range(H):
            pt_kt = []
            for kt in range(nkt):
                ptp = psum([P, TOK], "medb", 2, dtype=bf)
                for qt in range(2):
                    nc.tensor.transpose(
                        out=ptp[:, qt * P:(qt + 1) * P],
                        in_=probs[(h, qt)][:, kt * P:(kt + 1) * P],
                        identity=ident_b)
                pt = st([P, TOK], bf, "ptT", 12)
                cpy(pt, ptp)
                pt_kt.append(pt)
            avp = psum([64, TOK], "med", 2)
            for kt in range(nkt):
                nc.tensor.matmul(out=avp,
                                 lhsT=v_sb[kt][:, h * HD:(h + 1) * HD],
                                 rhs=pt_kt[kt],
                                 start=(kt == 0), stop=(kt == nkt - 1))
            cpy(avT[h // 2][(h % 2) * 64:(h % 2) * 64 + 64, :], avp)
        # wo + bias + residual
        r2 = []
        for me in range(ET):
            wo_sb = st([P, E], bf, "wo_sb", 3)
            nc.sync.dma_start(out=wo_sb, in_=d[wo_name][me, :, :])
            wop = psum([P, TOK], "big", 3)
            for ke in range(ET):
                nc.tensor.matmul(out=wop, lhsT=wo_sb[:, ke * P:(ke + 1) * P],
                                 rhs=avT[ke], start=(ke == 0), stop=(ke == ET - 1))
            t1 = st([P, TOK], f32, "t1", 3)
            nc.vector.tensor_scalar(out=t1, in0=wop, scalar1=bocol[me], scalar2=None,
                                    op0=ALU.add)
            r = st([P, TOK], f32, "r2", 9)
            nc.vector.tensor_tensor(out=r, in0=t1, in1=res_in[me], op=ALU.add)
            r2.append(r)
        return r2

    # ---- layers ----------------------------------------------------------
    for l in range(NL):
        gb = load_wcol(f"l{l}_gb", "gb", 2)      # [128, 8, 6]
        bov = load_wcol(f"l{l}_bov", "bov", 2)   # [128, 8, 3]
        b1c = load_wcol(f"l{l}_b1", "b1c", 2)    # [128, 32]

        def gcol(vec, col, ncols):
            return [vec[:, e, col:col + 1] for e in range(ET)]

        # pair all-gather of h
        cc_in = dr.tile([E, TOK], bf, tag="ccin", bufs=2, name="ccin")
        for e in range(ET):
            nc.sync.dma_start(out=cc_in[e * P:(e + 1) * P, :], in_=h_bf[e])
        cc_out = dr.tile([2 * E, TOK], bf, tag="ccout", bufs=2, name="ccout")
        nc.gpsimd.collective_compute(
            "AllGather", ALU.bypass, replica_groups=PAIRS,
            ins=[cc_in.opt()], outs=[cc_out.opt()])
        hg = [[], []]
        for half in range(2):
            for e in range(ET):
                t = st([P, TOK], bf, "hg", 2 * ET)
                nc.sync.dma_start(
                    out=t, in_=cc_out[half * E + e * P: half * E + (e + 1) * P, :])
                hg[half].append(t)

        # self-attn K/V over full 512 tokens
        wk = st([P, P], bf, "wbd", 6)
        nc.sync.dma_start(out=wk, in_=d[f"l{l}_swk"][:, :])
        wv = st([P, P], bf, "wbd", 6)
        nc.sync.dma_start(out=wv, in_=d[f"l{l}_swv"][:, :])
        kT = []
        for e in range(ET):
            kp = psum([P, T], "big", 3)
            for half in range(2):
                nc.tensor.matmul(out=kp[:, half * TOK:(half + 1) * TOK],
                                 lhsT=wk, rhs=hg[half][e], start=True, stop=True)
            k = st([P, T], bf, "kT", ET)
            cpy(k, kp)
            kT.append(k)
        v_sb = []
        for kt in range(KT):
            vp0 = psum([P, 512], "big", 3)
            vp1 = psum([P, 512], "big", 3)
            for e in range(ET):
                vp = vp0 if e < 4 else vp1
                nc.tensor.matmul(
                    out=vp[:, (e % 4) * P:(e % 4 + 1) * P],
                    lhsT=hg[kt // 2][e][:, (kt % 2) * P:(kt % 2 + 1) * P],
                    rhs=wv, start=True, stop=True)
            v = st([P, E], bf, "v_sb", KT)
            cpy(v[:, 0:512], vp0)
            cpy(v[:, 512:1024], vp1)
            v_sb.append(v)

        bo_s = gcol(bov, 0, 3)
        r2 = attention(l, h_bf, kT, v_sb, KT, True, f"l{l}_swo", resid, bo_s)
        resid, h_bf = layer_norm(r2, gcol(gb, 0, 6), gcol(gb, 1, 6), "ln0")

        # cross-attn K/V from encoder
        wk2 = st([P, P], bf, "wbd", 6)
        nc.sync.dma_start(out=wk2, in_=d[f"l{l}_cwk"][:, :])
        wv2 = st([P, P], bf, "wbd", 6)
        nc.sync.dma_start(out=wv2, in_=d[f"l{l}_cwv"][:, :])
        kTc = []
        for e in range(ET):
            kp = psum([P, S], "big", 3)
            nc.tensor.matmul(out=kp, lhsT=wk2, rhs=enc_sb[e], start=True, stop=True)
            k = st([P, S], bf, "kT", ET)
            cpy(k, kp)
            kTc.append(k)
        v_sbc = []
        for kt in range(KT):
            vp0 = psum([P, 512], "big", 3)
            vp1 = psum([P, 512], "big", 3)
            for e in range(ET):
                vp = vp0 if e < 4 else vp1
                nc.tensor.matmul(
                    out=vp[:, (e % 4) * P:(e % 4 + 1) * P],
                    lhsT=enc_sb[e][:, kt * P:(kt + 1) * P],
                    rhs=wv2, start=True, stop=True)
            v = st([P, E], bf, "v_sb", KT)
            cpy(v[:, 0:512], vp0)
            cpy(v[:, 512:1024], vp1)
            v_sbc.append(v)

        bo_c = gcol(bov, 1, 3)
        r2 = attention(l, h_bf, kTc, v_sbc, KT, False, f"l{l}_cwo", resid, bo_c)
        resid, h_bf = layer_norm(r2, gcol(gb, 2, 6), gcol(gb, 3, 6), "ln1")

        # FFN
        h1 = []
        for mf in range(FT):
            w1sb = st([P, E], bf, "w1sb", 4)
            nc.sync.dma_start(out=w1sb, in_=d[f"l{l}_w1"][mf, :, :])
            fp = psum([P, TOK], "big", 3)
            for ke in range(ET):
                nc.tensor.matmul(out=fp, lhsT=w1sb[:, ke * P:(ke + 1) * P],
                                 rhs=h_bf[ke], start=(ke == 0), stop=(ke == ET - 1))
            h1t = st([P, TOK], bf, "h1", FT + 2)
            nc.scalar.activation(out=h1t, in_=fp, func=AF.Relu,
                                 bias=b1c[:, mf:mf + 1], scale=1.0)
            h1.append(h1t)
        b2col = gcol(bov, 2, 3)
        r2 = []
        for me in range(ET):
            w2sb = st([P, FF], bf, "w2sb", 2)
            nc.sync.dma_start(out=w2sb, in_=d[f"l{l}_w2"][me, :, :])
            f2p = psum([P, TOK], "big", 3)
            for kf in range(FT):
                nc.tensor.matmul(out=f2p, lhsT=w2sb[:, kf * P:(kf + 1) * P],
                                 rhs=h1[kf], start=(kf == 0), stop=(kf == FT - 1))
            t1 = st([P, TOK], f32, "t1", 3)
            nc.vector.tensor_scalar(out=t1, in0=f2p, scalar1=b2col[me], scalar2=None,
                                    op0=ALU.add)
            r = st([P, TOK], f32, "r2", 9)
            nc.vector.tensor_tensor(out=r, in0=t1, in1=resid[me], op=ALU.add)
            r2.append(r)
        resid, h_bf = layer_norm(r2, gcol(gb, 4, 6), gcol(gb, 5, 6), "ln2")

    # ---- final all-gather + fc_out --------------------------------------
    cc2_in = dr.tile([E, TOK], bf, tag="cc2in", bufs=1, name="cc2in")
    for e in range(ET):
        nc.sync.dma_start(out=cc2_in[e * P:(e + 1) * P, :], in_=h_bf[e])
    cc2_out = dr.tile([NC_CORES * E, TOK], bf, tag="cc2out", bufs=1,
                      addr_space="Shared", name="cc2out")
    nc.gpsimd.collective_compute(
        "AllGather", ALU.bypass, replica_groups=ALL8,
        ins=[cc2_in.opt()], outs=[cc2_out.opt()])
    hall = []
    for e in range(ET):
        t = st([P, NTOK_ALL], bf, "hall", ET)
        for r in range(NC_CORES):
            nc.sync.dma_start(
                out=t[:, r * TOK:(r + 1) * TOK],
                in_=cc2_out[r * E + e * P: r * E + (e + 1) * P, :])
        hall.append(t)
    fcbc = load_wcol("fcb", "fcbc", 1)  # [128, 32]
    for vt in range(FT):
        fcsb = st([P, E], bf, "w1sb", 4)
        nc.sync.dma_start(out=fcsb, in_=d["fcw"][vt, :, :])
        for ch in range(4):
            lp = psum([P, 512], "big", 3)
            for ke in range(ET):
                nc.tensor.matmul(out=lp, lhsT=fcsb[:, ke * P:(ke + 1) * P],
                                 rhs=hall[ke][:, ch * 512:(ch + 1) * 512],
                                 start=(ke == 0), stop=(ke == ET - 1))
            ls = st([P, 512], f32, "ls", 4)
            if vt % 2:
                nc.vector.tensor_scalar(out=ls, in0=lp, scalar1=fcbc[:, vt:vt + 1],
                                        scalar2=None, op0=ALU.add)
            else:
                nc.scalar.activation(out=ls, in_=lp, func=AF.Identity,
                                     bias=fcbc[:, vt:vt + 1], scale=1.0)
            nc.sync.dma_start(out=out_d[vt * P:(vt + 1) * P, ch * 512:(ch + 1) * 512],
                              in_=ls)


# ------------------------------------------------------------- host side ---

def _bd(w, scale=1.0):
    """[64,64] weight -> [128,128] block-diag of w.T (2 heads per K-tile)."""
    wt = (np.asarray(w, np.float32).T * scale)
    out = np.zeros((P, P), np.float32)
    out[:HD, :HD] = wt
    out[HD:, HD:] = wt
    return out.astype(BF16)


def _tiled(w, mt, kt):
    """[M, K] f32 -> [M/128, 128(k-part), K] bf16 with A[m,p,k,f] = w[m*128+f, k*128+p]."""
    M, K = w.shape
    a = np.asarray(w, np.float32).reshape(mt, P, kt, P).transpose(0, 3, 2, 1)
    return np.ascontiguousarray(a.reshape(mt, P, kt * P)).astype(BF16)


def prepare_in_maps(x, enc_out, src_mask, tgt_mask, params):
    x = np.asarray(x)
    enc_out = np.asarray(enc_out, np.float32)
    src_mask = np.asarray(src_mask, np.float32)
    tgt_mask = np.asarray(tgt_mask, np.float32)

    scale = 1.0 / np.sqrt(np.float32(E))
    shared = {}
    shared["wemb"] = np.asarray(params["word_emb"], np.float32).astype(BF16)
    fcw = np.asarray(params["fc_w"], np.float32)
    fcw_pad = np.zeros((VPAD, E), np.float32)
    fcw_pad[:V] = fcw
    fcb_pad = np.zeros((VPAD,), np.float32)
    fcb_pad[:V] = np.asarray(params["fc_b"], np.float32)
    pos = np.asarray(params["pos_emb"], np.float32)

    for l, lp in enumerate(params["layers"]):
        for pre, key in (("s", "self"), ("c", "cross")):
            ap = lp[key]
            shared[f"l{l}_{pre}wq"] = _bd(ap["wq"], scale)
            shared[f"l{l}_{pre}wk"] = _bd(ap["wk"])
            shared[f"l{l}_{pre}wv"] = _bd(ap["wv"])
            shared[f"l{l}_{pre}wo"] = _tiled(np.asarray(ap["wo"], np.float32), ET, ET)
        shared[f"l{l}_w1"] = _tiled(np.asarray(lp["w1"], np.float32), FT, ET)
        shared[f"l{l}_w2"] = _tiled(np.asarray(lp["w2"], np.float32), ET, FT)
        shared[f"l{l}_gb"] = np.ascontiguousarray(np.stack(
            [np.asarray(lp[k], np.float32) for k in
             ("norm_g", "norm_b", "n1_g", "n1_b", "n2_g", "n2_b")], axis=1))
        shared[f"l{l}_bov"] = np.ascontiguousarray(np.stack(
            [np.asarray(lp["self"]["bo"], np.float32),
             np.asarray(lp["cross"]["bo"], np.float32),
             np.asarray(lp["b2"], np.float32)], axis=1))
        shared[f"l{l}_b1"] = np.asarray(lp["b1"], np.float32)

    in_maps = []
    for c in range(NC_CORES):
        b, half = c // 2, c % 2
        m = dict(shared)
        m["idx"] = x[b, half * TOK:(half + 1) * TOK].astype(np.int32)
        m["pos_t"] = np.ascontiguousarray(
            pos[half * TOK:(half + 1) * TOK].T.astype(np.float32))
        m["enc_t"] = np.ascontiguousarray(enc_out[b].T).astype(BF16)
        m["tmask"] = np.ascontiguousarray(
            tgt_mask[b, 0, half * TOK:(half + 1) * TOK, :])
        m["smask"] = np.ascontiguousarray(src_mask[b, 0, 0, :][None, :])
        m["fcw"] = _tiled(fcw_pad[c * VS:(c + 1) * VS], FT, ET)
        m["fcb"] = np.ascontiguousarray(fcb_pad[c * VS:(c + 1) * VS])
        in_maps.append(m)
    return in_maps


def assemble_output(results):
    """results: list of per-core {'out': [VS, 2048] f32} -> [N, T, V] f32."""
    full = np.empty((N, T, VPAD), np.float32)
    for c in range(NC_CORES):
        oc = np.asarray(results[c]["out"]).reshape(VS, NC_CORES, TOK)
        for r in range(NC_CORES):
            b, half = r // 2, r % 2
            full[b, half * TOK:(half + 1) * TOK, c * VS:(c + 1) * VS] = \
                oc[:, r, :].T
    return np.ascontiguousarray(full[:, :, :V])


def kernel(x, enc_out, src_mask, tgt_mask, params, trace=False):
    if "nc" not in _CACHE:
        _CACHE["nc"] = build_nc()
    nc = _CACHE["nc"]
    in_maps = prepare_in_maps(x, enc_out, src_mask, tgt_mask, params)
    res = run_bass_kernel_spmd(nc, in_maps, list(range(NC_CORES)), trace=False)
    return assemble_output(res.results)


def run_timed(in_maps, nc=None, iters=8):
    """Mirror bass2jax.run_bass_via_pjrt's shard_map path without output
    donation so the staged inputs can be re-executed and timed.
    Returns (results_list, per_iter_wall_seconds)."""
    import time
    import jax
    from jax.sharding import Mesh, PartitionSpec
    from jax.experimental.shard_map import shard_map
    from concourse import bass2jax as b2j
    from concourse import mybir as _mb

    if nc is None:
        if "nc" not in _CACHE:
            _CACHE["nc"] = build_nc()
        nc = _CACHE["nc"]
    b2j.install_neuronx_cc_hook()
    n_cores = NC_CORES
    partition_name = nc.partition_id_tensor.name if nc.partition_id_tensor else None
    in_names, out_names, out_avals = [], [], []
    for alloc in nc.m.functions[0].allocations:
        if not isinstance(alloc, _mb.MemoryLocationSet):
            continue
        name = alloc.memorylocations[0].name
        if alloc.kind == "ExternalInput":
            if name != partition_name:
                in_names.append(name)
        elif alloc.kind == "ExternalOutput":
            out_names.append(name)
            out_avals.append(jax.core.ShapedArray(
                tuple(alloc.tensor_shape), _mb.dt.np(alloc.dtype)))
    n_params = len(in_names)
    all_names = in_names + out_names
    if partition_name is not None:
        all_names.append(partition_name)

    def _body(*args):
        operands = list(args)
        # outputs are fully written by the kernel; pass fresh zeros anyway
        if partition_name is not None:
            operands.append(b2j.partition_id_tensor())
        outs = b2j._bass_exec_p.bind(
            *operands,
            out_avals=tuple(out_avals),
            in_names=tuple(all_names),
            out_names=tuple(out_names),
            lowering_input_output_aliases=(),
            sim_require_finite=True,
            sim_require_nnan=True,
            nc=nc,
        )
        return tuple(outs)

    devices = jax.devices()[:n_cores]
    mesh = Mesh(np.array(devices), ("core",))
    nouts = len(out_names)
    sharded = jax.jit(
        shard_map(_body, mesh=mesh,
                  in_specs=(PartitionSpec("core"),) * (n_params + nouts),
                  out_specs=(PartitionSpec("core"),) * nouts,
                  check_rep=False),
        keep_unused=True)
    concat_in = [np.concatenate([np.asarray(in_maps[c][nm]) for c in range(n_cores)],
                                axis=0) for nm in in_names]
    concat_zeros = [np.zeros((n_cores * a.shape[0], *a.shape[1:]), a.dtype)
                    for a in out_avals]
    from jax.sharding import NamedSharding
    sh = [NamedSharding(mesh, PartitionSpec("core"))] * (n_params + nouts)
    staged = [jax.device_put(a, s) for a, s in zip(concat_in + concat_zeros, sh)]
    out = sharded(*staged)
    jax.block_until_ready(out)
    times = []
    for _ in range(iters):
        t0 = time.perf_counter()
        out = sharded(*staged)
        jax.block_until_ready(out)
        times.append(time.perf_counter() - t0)
    results = [
        {nm: np.asarray(out[i]).reshape(n_cores, *out_avals[i].shape)[c]
         for i, nm in enumerate(out_names)}
        for c in range(n_cores)
    ]
    return results, times


# revision 17
# speedup vs baseline: 11605.9334x; 11605.9334x over previous
"""Trainium2 Bass kernel for nn_Decoder (4-layer post-LN transformer decoder).

Sharding: 8 cores = 4 batches x 2 sequence-halves (256 tokens each).
Activations on-chip live transposed [E, tok]. Per layer a pair-wise
AllGather shares the hidden state so self-attn K/V sees the full 512-token
sequence. fc_out is vocab-sharded 8 ways after a full AllGather.
Compute: bf16 matmul operands, f32 accumulation / softmax / LN.
"""

import numpy as np
import ml_dtypes
from contextlib import ExitStack

import concourse.bass as bass
import concourse.tile as tile
from concourse import mybir
from concourse.bass_utils import run_bass_kernel_spmd
from concourse.masks import make_identity

BF16 = ml_dtypes.bfloat16

# Problem dims (hardcoded)
N, T, S, E, H, V, L, FF, NL = 4, 512, 512, 1024, 16, 32000, 1024, 4096, 4
HD = E // H          # 64
NC_CORES = 8
TOK = 256            # tokens per core
VS = 4096            # vocab shard (padded 32768 / 8)
VPAD = VS * NC_CORES
ET = E // 128        # 8
FT = FF // 128       # 32
KT = S // 128        # 4
P = 128
NTOK_ALL = NC_CORES * TOK  # 2048

DT = mybir.dt
AX = mybir.AxisListType
ALU = mybir.AluOpType
AF = mybir.ActivationFunctionType

PAIRS = [[0, 1], [2, 3], [4, 5], [6, 7]]
ALL8 = [list(range(NC_CORES))]

_CACHE = {}
LAST_RESULTS = None  # BassKernelResults of most recent run (for test.py)


# ---------------------------------------------------------------- builder ---

def _inp(nc, d, name, shape, dtype):
    d[name] = nc.declare_dram_parameter(name, list(shape), dtype, isOutput=False)
    return d[name]


def build_nc():
    nc = bass.Bass(num_devices=NC_CORES)
    f32, bf, i32 = DT.float32, DT.bfloat16, DT.int32
    d = {}
    _inp(nc, d, "idx", [TOK], i32)
    _inp(nc, d, "pos_t", [E, TOK], f32)
    _inp(nc, d, "enc_t", [E, S], bf)
    _inp(nc, d, "tmask", [TOK, T], f32)
    _inp(nc, d, "smask", [1, S], f32)
    _inp(nc, d, "wemb", [V, E], bf)
    _inp(nc, d, "fcw", [FT, P, E], bf)
    _inp(nc, d, "fcb", [VS], f32)
    for l in range(NL):
        for w in ("swq", "swk", "swv", "cwq", "cwk", "cwv"):
            _inp(nc, d, f"l{l}_{w}", [P, P], bf)
        _inp(nc, d, f"l{l}_swo", [ET, P, E], bf)
        _inp(nc, d, f"l{l}_cwo", [ET, P, E], bf)
        _inp(nc, d, f"l{l}_w1", [FT, P, E], bf)
        _inp(nc, d, f"l{l}_w2", [ET, P, FF], bf)
        _inp(nc, d, f"l{l}_gb", [E, 6], f32)   # columns: g0 b0 g1 b1 g2 b2
        _inp(nc, d, f"l{l}_bov", [E, 3], f32)  # columns: bo_self bo_cross b2
        _inp(nc, d, f"l{l}_b1", [FF], f32)
    out_d = nc.declare_dram_parameter("out", [VS, NTOK_ALL], f32, isOutput=True)

    with tile.TileContext(nc) as tc:
        with ExitStack() as ctx:
            _build_program(ctx, tc, d, out_d)
    _spill_excess_waits(nc)
    return nc


def _spill_excess_waits(nc):
    """This walrus build caps sync waits at 1 per instruction (2 for
    EventSemaphore). Move extra waits onto standalone EventSemaphore
    instructions inserted just before the offender on the same engine —
    serial waits on the same queue are semantically identical to a
    combined wait list."""
    n_new = [0]

    def mk_ev(engine, waits, debug):
        n_new[0] += 1
        ev = mybir.InstEventSemaphore(
            name=f"EVW-{n_new[0]}", engine=engine, ins=[], outs=[],
            sync_info=mybir.SyncInfo(on_wait=list(waits), on_update=[]),
            debug=debug)
        return ev

    for fn in nc.m.functions:
        for blk in fn.blocks:
            out = []
            changed = False
            for inst in blk.instructions:
                si = inst.sync_info
                cap = 2 if isinstance(inst, mybir.InstEventSemaphore) else 1
                if si is not None and len(si.on_wait) > cap:
                    waits = list(si.on_wait)
                    extra, keep = waits[:-cap], waits[-cap:]
                    for i in range(0, len(extra), 2):
                        out.append(mk_ev(inst.engine, extra[i:i + 2], inst.debug))
                    inst.sync_info = mybir.SyncInfo(
                        on_wait=keep, on_update=list(si.on_update))
                    changed = True
                out.append(inst)
            if changed:
                blk.instructions = out


def _build_program(ctx, tc, d, out_d):
    nc = tc.nc
    f32, bf = DT.float32, DT.bfloat16

    pp = ctx.enter_context(tc.tile_pool(name="pp", bufs=1, space="PSUM"))
    sb = ctx.enter_context(tc.tile_pool(name="sb", bufs=1))
    dr = ctx.enter_context(tc.tile_pool(name="dr", bufs=1, space="DRAM"))

    def psum(shape, tag, bufs, dtype=f32):
        return pp.tile(shape, dtype, tag=tag, bufs=bufs, name=tag)

    def st(shape, dtype, tag, bufs):
        return sb.tile(shape, dtype, tag=tag, bufs=bufs, name=tag)

    cp_i = [0]

    def cpy(out, in_):
        # alternate bulk PSUM->SBUF copies between ACT and DVE
        cp_i[0] += 1
        if cp_i[0] % 2:
            nc.scalar.copy(out, in_)
        else:
            nc.vector.tensor_copy(out, in_)

    # ---- constants -------------------------------------------------------
    ident_f = st([P, P], f32, "ident_f", 1)
    make_identity(nc, ident_f[:, :])
    ident_b = st([P, P], bf, "ident_b", 1)
    nc.vector.tensor_copy(ident_b, ident_f)
    ones_f = st([P, 1], f32, "ones_f", 1)
    nc.vector.memset(ones_f, 1.0)
    ones_b = st([P, 1], bf, "ones_b", 1)
    nc.vector.memset(ones_b, 1.0)
    ones1_f = st([1, P], f32, "ones1_f", 1)
    nc.vector.memset(ones1_f, 1.0)
    ones1_b = st([1, P], bf, "ones1_b", 1)
    nc.vector.memset(ones1_b, 1.0)
    eps_t = st([1, 1], f32, "eps_t", 1)
    nc.vector.memset(eps_t, 1e-5)

    # masks
    madd_self = []
    for qt in range(2):
        tmt = st([P, T], f32, "tm", 2)
        nc.sync.dma_start(out=tmt, in_=d["tmask"][qt * P:(qt + 1) * P, :])
        ms = st([P, T], bf, f"madd_self{qt}", 1)
        nc.vector.tensor_scalar(out=ms, in0=tmt, scalar1=1e20, scalar2=1e20,
                                op0=ALU.mult, op1=ALU.subtract)
        madd_self.append(ms)
    smr = st([1, S], f32, "smr", 1)
    nc.sync.dma_start(out=smr, in_=d["smask"][:, :])
    madd_cross = st([1, S], bf, "madd_cross", 1)
    nc.vector.tensor_scalar(out=madd_cross, in0=smr, scalar1=1e20, scalar2=1e20,
                            op0=ALU.mult, op1=ALU.subtract)

    # encoder (transposed) resident bf16
    enc_sb = []
    for e in range(ET):
        t = st([P, S], bf, "enc_sb", ET)
        nc.sync.dma_start(out=t, in_=d["enc_t"][e * P:(e + 1) * P, :])
        enc_sb.append(t)

    # ---- embedding -------------------------------------------------------
    resid = []
    h_bf = []
    for e in range(ET):
        resid.append(st([P, TOK], f32, "resid", 9))
        h_bf.append(st([P, TOK], bf, "hbf", 10))
    pos_sb = []
    for e in range(ET):
        ps = st([P, TOK], f32, "xc", 10)
        nc.sync.dma_start(out=ps, in_=d["pos_t"][e * P:(e + 1) * P, :])
        pos_sb.append(ps)
    for tb in range(2):
        it = st([P, 1], DT.int32, "idx", 2)
        nc.sync.dma_start(out=it, in_=d["idx"][tb * P:(tb + 1) * P, None])
        g = st([P, E], bf, "gath", 2)
        nc.gpsimd.indirect_dma_start(
            out=g, out_offset=None, in_=d["wemb"][:, :],
            in_offset=bass.IndirectOffsetOnAxis(ap=it[:, :1], axis=0))
        for e in range(ET):
            tp = psum([P, P], "medb", 2, dtype=bf)
            nc.tensor.transpose(out=tp, in_=g[:, e * P:(e + 1) * P], identity=ident_b)
            nc.vector.tensor_tensor(out=resid[e][:, tb * P:(tb + 1) * P],
                                    in0=tp, in1=pos_sb[e][:, tb * P:(tb + 1) * P],
                                    op=ALU.add)
    for e in range(ET):
        nc.scalar.copy(h_bf[e], resid[e])

    # ---- helpers ---------------------------------------------------------
    def load_wcol(name, tag, bufs):
        # [dim] f32 -> SBUF [128, dim/128]; [dim, c] f32 -> SBUF [128, dim/128, c]
        src = d[name]
        if len(src.shape) == 1:
            t = st([P, src.shape[0] // P], f32, tag, bufs)
            nc.sync.dma_start(out=t, in_=src.rearrange("(a p) -> p a", p=P))
        else:
            t = st([P, src.shape[0] // P, src.shape[1]], f32, tag, bufs)
            nc.sync.dma_start(out=t, in_=src.rearrange("(a p) c -> p a c", p=P))
        return t

    def layer_norm(r2, gcol, bcol, lname):
        """r2: list of 8 [P,TOK] f32 SBUF tiles. gcol/bcol: [P,1] APs per etile.
        Returns (resid_new, ybf) lists."""
        sums = psum([1, TOK], "stat", 1)
        for e in range(ET):
            nc.tensor.matmul(out=sums, lhsT=ones_f, rhs=r2[e],
                             start=(e == 0), stop=(e == ET - 1))
        mu_row = st([1, TOK], f32, "mu_row", 2)
        nc.scalar.mul(mu_row, sums, 1.0 / E)
        Mu = psum([P, TOK], "med", 2)
        nc.tensor.matmul(out=Mu, lhsT=ones1_f, rhs=mu_row, start=True, stop=True)
        xc = []
        ssq = psum([1, TOK], "stat", 1)
        for e in range(ET):
            x = st([P, TOK], f32, "xc", 10)
            nc.vector.tensor_tensor(out=x, in0=r2[e], in1=Mu, op=ALU.subtract)
            xc.append(x)
            sq = st([P, TOK], bf, "sq", 4)
            nc.vector.tensor_tensor(out=sq, in0=x, in1=x, op=ALU.mult)
            nc.tensor.matmul(out=ssq, lhsT=ones_b, rhs=sq,
                             start=(e == 0), stop=(e == ET - 1))
        std_row = st([1, TOK], f32, "std_row", 2)
        nc.scalar.activation(out=std_row, in_=ssq, func=AF.Sqrt,
                             bias=eps_t[:, :], scale=1.0 / E)
        rstd_row = st([1, TOK], f32, "rstd_row", 2)
        nc.vector.reciprocal(rstd_row, std_row)
        Rstd = psum([P, TOK], "med", 2)
        nc.tensor.matmul(out=Rstd, lhsT=ones1_f, rhs=rstd_row, start=True, stop=True)
        rn, yb = [], []
        for e in range(ET):
            xn = st([P, TOK], f32, "xn", 3)
            nc.vector.tensor_tensor(out=xn, in0=xc[e], in1=Rstd, op=ALU.mult)
            r = st([P, TOK], f32, "resid", 9)
            nc.vector.tensor_scalar(out=r, in0=xn, scalar1=gcol[e], scalar2=bcol[e],
                                    op0=ALU.mult, op1=ALU.add)
            y = st([P, TOK], bf, "hbf", 10)
            nc.scalar.copy(y, r)
            rn.append(r)
            yb.append(y)
        return rn, yb

    def attention(l, q_src, kT, v_sb, nkt, self_mask, wo_name, res_in, bocol):
        """q_src: 8 bf16 [P,TOK] tiles. kT: 8 [P, nkt*128] bf16. v_sb: nkt
        [P,E] bf16. Returns r2 list (resid + attn out + bo), pre-LN."""
        wq = st([P, P], bf, "wbd", 6)
        nc.sync.dma_start(out=wq, in_=d[f"l{l}_{'swq' if self_mask else 'cwq'}"][:, :])
        qT = []
        for e in range(ET):
            qp = psum([P, TOK], "big", 3)
            nc.tensor.matmul(out=qp, lhsT=wq, rhs=q_src[e], start=True, stop=True)
            q = st([P, TOK], bf, "qT", ET)
            cpy(q, qp)
            qT.append(q)
        nk = nkt * P
        probs = {}
        for h in range(H):
            ep_pair, r0 = h // 2, (h % 2) * 64
            for qt in range(2):
                ep = psum([P, nk], "big", 3)
                nc.tensor.matmul(
                    out=ep,
                    lhsT=qT[ep_pair][r0:r0 + 64, qt * P:(qt + 1) * P],
                    rhs=kT[ep_pair][r0:r0 + 64, :],
                    start=True, stop=False)
                if self_mask:
                    nc.tensor.matmul(out=ep, lhsT=ident_b, rhs=madd_self[qt],
                                     start=False, stop=True)
                else:
                    nc.tensor.matmul(out=ep, lhsT=ones1_b, rhs=madd_cross,
                                     start=False, stop=True)
                mx = st([P, 1], f32, "mx", 6)
                nc.vector.reduce_max(mx, ep, axis=AX.X, negate=True)
                pr = st([P, nk], bf, "probs", 6)
                sm = st([P, 1], f32, "sm", 6)
                nc.scalar.activation(out=pr, in_=ep, func=AF.Exp,
                                     bias=mx, scale=1.0, accum_out=sm)
                rc = st([P, 1], f32, "rc", 6)
                nc.vector.reciprocal(rc, sm)
                nc.vector.tensor_scalar_mul(out=pr, in0=pr, scalar1=rc)
                probs[(h, qt)] = pr
        # transpose probs -> [k, q] then av
        avT = [st([P, TOK], bf, "avT", ET) for _ in range(ET)]
        for h in range(H):
            pt_kt = []
            for kt in range(nkt):
                ptp = psum([P, TOK], "medb", 2, dtype=bf)
                for qt in range(2):
                    nc.tensor.transpose(
                        out=ptp[:, qt * P:(qt + 1) * P],
                        in_=probs[(h, qt)][:, kt * P:(kt + 1) * P],
                        identity=ident_b)
                pt = st([P, TOK], bf, "ptT", 12)
                cpy(pt, ptp)
                pt_kt.append(pt)
            avp = psum([64, TOK], "med", 2)
            for kt in range(nkt):
                nc.tensor.matmul(out=avp,
                                 lhsT=v_sb[kt][:, h * HD:(h + 1) * HD],
                                 rhs=pt_kt[kt],
                                 start=(kt == 0), stop=(kt == nkt - 1))
            cpy(avT[h // 2][(h % 2) * 64:(h % 2) * 64 + 64, :], avp)
        # wo + bias + residual
        r2 = []
        for me in range(ET):
            wo_sb = st([P, E], bf, "wo_sb", 3)
            nc.sync.dma_start(out=wo_sb, in_=d[wo_name][me, :, :])
            wop = psum([P, TOK], "big", 3)
            for ke in range(ET):
                nc.tensor.matmul(out=wop, lhsT=wo_sb[:, ke * P:(ke + 1) * P],
                                 rhs=avT[ke], start=(ke == 0), stop=(ke == ET - 1))
            t1 = st([P, TOK], f32, "t1", 3)
            nc.vector.tensor_scalar(out=t1, in0=wop, scalar1=bocol[me], scalar2=None,
                                    op0=ALU.add)
            r = st([P, TOK], f32, "r2", 9)
            nc.vector.tensor_tensor(out=r, in0=t1, in1=res_in[me], op=ALU.add)
            r2.append(r)
        return r2

    # ---- layers ----------------------------------------------------------
    for l in range(NL):
        gb = load_wcol(f"l{l}_gb", "gb", 2)      # [128, 8, 6]
        bov = load_wcol(f"l{l}_bov", "bov", 2)   # [128, 8, 3]
        b1c = load_wcol(f"l{l}_b1", "b1c", 2)    # [128, 32]

        def gcol(vec, col, ncols):
            return [vec[:, e, col:col + 1] for e in range(ET)]

        # pair all-gather of h
        cc_in = dr.tile([E, TOK], bf, tag="ccin", bufs=2, name="ccin")
        for e in range(ET):
            nc.sync.dma_start(out=cc_in[e * P:(e + 1) * P, :], in_=h_bf[e])
        cc_out = dr.tile([2 * E, TOK], bf, tag="ccout", bufs=2, name="ccout")
        nc.gpsimd.collective_compute(
            "AllGather", ALU.bypass, replica_groups=PAIRS,
            ins=[cc_in.opt()], outs=[cc_out.opt()])
        hg = [[], []]
        for half in range(2):
            for e in range(ET):
                t = st([P, TOK], bf, "hg", 2 * ET)
                nc.sync.dma_start(
                    out=t, in_=cc_out[half * E + e * P: half * E + (e + 1) * P, :])
                hg[half].append(t)

        # self-attn K/V over full 512 tokens
        wk = st([P, P], bf, "wbd", 6)
        nc.sync.dma_start(out=wk, in_=d[f"l{l}_swk"][:, :])
        wv = st([P, P], bf, "wbd", 6)
        nc.sync.dma_start(out=wv, in_=d[f"l{l}_swv"][:, :])
        kT = []
        for e in range(ET):
            kp = psum([P, T], "big", 3)
            for half in range(2):
                nc.tensor.matmul(out=kp[:, half * TOK:(half + 1) * TOK],
                                 lhsT=wk, rhs=hg[half][e], start=True, stop=True)
            k = st([P, T], bf, "kT", ET)
            cpy(k, kp)
            kT.append(k)
        v_sb = []
        for kt in range(KT):
            vp0 = psum([P, 512], "big", 3)
            vp1 = psum([P, 512], "big", 3)
            for e in range(ET):
                vp = vp0 if e < 4 else vp1
                nc.tensor.matmul(
                    out=vp[:, (e % 4) * P:(e % 4 + 1) * P],
                    lhsT=hg[kt // 2][e][:, (kt % 2) * P:(kt % 2 + 1) * P],
                    rhs=wv, start=True, stop=True)
            v = st([P, E], bf, "v_sb", KT)
            cpy(v[:, 0:512], vp0)
            cpy(v[:, 512:1024], vp1)
            v_sb.append(v)

        bo_s = gcol(bov, 0, 3)
        r2 = attention(l, h_bf, kT, v_sb, KT, True, f"l{l}_swo", resid, bo_s)
        resid, h_bf = layer_norm(r2, gcol(gb, 0, 6), gcol(gb, 1, 6), "ln0")

        # cross-attn K/V from encoder
        wk2 = st([P, P], bf, "wbd", 6)
        nc.sync.dma_start(out=wk2, in_=d[f"l{l}_cwk"][:, :])
        wv2 = st([P, P], bf, "wbd", 6)
        nc.sync.dma_start(out=wv2, in_=d[f"l{l}_cwv"][:, :])
        kTc = []
        for e in range(ET):
            kp = psum([P, S], "big", 3)
            nc.tensor.matmul(out=kp, lhsT=wk2, rhs=enc_sb[e], start=True, stop=True)
            k = st([P, S], bf, "kT", ET)
            cpy(k, kp)
            kTc.append(k)
        v_sbc = []
        for kt in range(KT):
            vp0 = psum([P, 512], "big", 3)
            vp1 = psum([P, 512], "big", 3)
            for e in range(ET):
                vp = vp0 if e < 4 else vp1
                nc.tensor.matmul(
                    out=vp[:, (e % 4) * P:(e % 4 + 1) * P],
                    lhsT=enc_sb[e][:, kt * P:(kt + 1) * P],
                    rhs=wv2, start=True, stop=True)
            v = st([P, E], bf, "v_sb", KT)
            cpy(v[:, 0:512], vp0)
            cpy(v[:, 512:1024], vp1)
            v_sbc.append(v)

        bo_c = gcol(bov, 1, 3)
        r2 = attention(l, h_bf, kTc, v_sbc, KT, False, f"l{l}_cwo", resid, bo_c)
        resid, h_bf = layer_norm(r2, gcol(gb, 2, 6), gcol(gb, 3, 6), "ln1")

        # FFN
        h1 = []
        for mf in range(FT):
            w1sb = st([P, E], bf, "w1sb", 4)
            nc.sync.dma_start(out=w1sb, in_=d[f"l{l}_w1"][mf, :, :])
            fp = psum([P, TOK], "big", 3)
            for ke in range(ET):
                nc.tensor.matmul(out=fp, lhsT=w1sb[:, ke * P:(ke + 1) * P],
                                 rhs=h_bf[ke], start=(ke == 0), stop=(ke == ET - 1))
            h1t = st([P, TOK], bf, "h1", FT + 2)
            nc.scalar.activation(out=h1t, in_=fp, func=AF.Relu,
                                 bias=b1c[:, mf:mf + 1], scale=1.0)
            h1.append(h1t)
        b2col = gcol(bov, 2, 3)
        r2 = []
        for me in range(ET):
            w2sb = st([P, FF], bf, "w2sb", 2)
            nc.sync.dma_start(out=w2sb, in_=d[f"l{l}_w2"][me, :, :])
            f2p = psum([P, TOK], "big", 3)
            for kf in range(FT):
                nc.tensor.matmul(out=f2p, lhsT=w2sb[:, kf * P:(kf + 1) * P],
                                 rhs=h1[kf], start=(kf == 0), stop=(kf == FT - 1))
            t1 = st([P, TOK], f32, "t1", 3)
            nc.vector.tensor_scalar(out=t1, in0=f2p, scalar1=b2col[me], scalar2=None,
                                    op0=ALU.add)
            r = st([P, TOK], f32, "r2", 9)
            nc.vector.tensor_tensor(out=r, in0=t1, in1=resid[me], op=ALU.add)
            r2.append(r)
        resid, h_bf = layer_norm(r2, gcol(gb, 4, 6), gcol(gb, 5, 6), "ln2")

    # ---- final all-gather + fc_out --------------------------------------
    cc2_in = dr.tile([E, TOK], bf, tag="cc2in", bufs=1, name="cc2in")
    for e in range(ET):
        nc.sync.dma_start(out=cc2_in[e * P:(e + 1) * P, :], in_=h_bf[e])
    cc2_out = dr.tile([NC_CORES * E, TOK], bf, tag="cc2out", bufs=1,
                      addr_space="Shared", name="cc2out")
    nc.gpsimd.collective_compute(
        "AllGather", ALU.bypass, replica_groups=ALL8,
        ins=[cc2_in.opt()], outs=[cc2_out.opt()])
    hall = []
    for e in range(ET):
        t = st([P, NTOK_ALL], bf, "hall", ET)
        for r in range(NC_CORES):
            nc.sync.dma_start(
                out=t[:, r * TOK:(r + 1) * TOK],
                in_=cc2_out[r * E + e * P: r * E + (e + 1) * P, :])
        hall.append(t)
    fcbc = load_wcol("fcb", "fcbc", 1)  # [128, 32]
    for vt in range(FT):
        fcsb = st([P, E], bf, "w1sb", 4)
        nc.sync.dma_start(out=fcsb, in_=d["fcw"][vt, :, :])
        for ch in range(4):
            lp = psum([P, 512], "big", 3)
            for ke in range(ET):
                nc.tensor.matmul(out=lp, lhsT=fcsb[:, ke * P:(ke + 1) * P],
                                 rhs=hall[ke][:, ch * 512:(ch + 1) * 512],
                                 start=(ke == 0), stop=(ke == ET - 1))
            ls = st([P, 512], f32, "ls", 4)
            if vt % 2:
                nc.vector.tensor_scalar(out=ls, in0=lp, scalar1=fcbc[:, vt:vt + 1],
                                        scalar2=None, op0=ALU.add)
            else:
                nc.scalar.activation(out=ls, in_=lp, func=AF.Identity,
                                     bias=fcbc[:, vt:vt + 1], scale=1.0)
            nc.sync.dma_start(out=out_d[vt * P:(vt + 1) * P, ch * 512:(ch + 1) * 512],
                              in_=ls)


# ------------------------------------------------------------- host side ---

def _bd(w, scale=1.0):
    """[64,64] weight -> [128,128] block-diag of w.T (2 heads per K-tile)."""
    wt = (np.asarray(w, np.float32).T * scale)
    out = np.zeros((P, P), np.float32)
    out[:HD, :HD] = wt
    out[HD:, HD:] = wt
    return out.astype(BF16)


def _tiled(w, mt, kt):
    """[M, K] f32 -> [M/128, 128(k-part), K] bf16 with A[m,p,k,f] = w[m*128+f, k*128+p]."""
    M, K = w.shape
    a = np.asarray(w, np.float32).reshape(mt, P, kt, P).transpose(0, 3, 2, 1)
    return np.ascontiguousarray(a.reshape(mt, P, kt * P)).astype(BF16)


def prepare_in_maps(x, enc_out, src_mask, tgt_mask, params):
    x = np.asarray(x)
    enc_out = np.asarray(enc_out, np.float32)
    src_mask = np.asarray(src_mask, np.float32)
    tgt_mask = np.asarray(tgt_mask, np.float32)

    scale = 1.0 / np.sqrt(np.float32(E))
    shared = {}
    shared["wemb"] = np.asarray(params["word_emb"], np.float32).astype(BF16)
    fcw = np.asarray(params["fc_w"], np.float32)
    fcw_pad = np.zeros((VPAD, E), np.float32)
    fcw_pad[:V] = fcw
    fcb_pad = np.zeros((VPAD,), np.float32)
    fcb_pad[:V] = np.asarray(params["fc_b"], np.float32)
    pos = np.asarray(params["pos_emb"], np.float32)

    for l, lp in enumerate(params["layers"]):
        for pre, key in (("s", "self"), ("c", "cross")):
            ap = lp[key]
            shared[f"l{l}_{pre}wq"] = _bd(ap["wq"], scale)
            shared[f"l{l}_{pre}wk"] = _bd(ap["wk"])
            shared[f"l{l}_{pre}wv"] = _bd(ap["wv"])
            shared[f"l{l}_{pre}wo"] = _tiled(np.asarray(ap["wo"], np.float32), ET, ET)
        shared[f"l{l}_w1"] = _tiled(np.asarray(lp["w1"], np.float32), FT, ET)
        shared[f"l{l}_w2"] = _tiled(np.asarray(lp["w2"], np.float32), ET, FT)
        shared[f"l{l}_gb"] = np.ascontiguousarray(np.stack(
            [np.asarray(lp[k], np.float32) for k in
             ("norm_g", "norm_b", "n1_g", "n1_b", "n2_g", "n2_b")], axis=1))
        shared[f"l{l}_bov"] = np.ascontiguousarray(np.stack(
            [np.asarray(lp["self"]["bo"], np.float32),
             np.asarray(lp["cross"]["bo"], np.float32),
             np.asarray(lp["b2"], np.float32)], axis=1))
        shared[f"l{l}_b1"] = np.asarray(lp["b1"], np.float32)

    in_maps = []
    for c in range(NC_CORES):
        b, half = c // 2, c % 2
        m = dict(shared)
        m["idx"] = x[b, half * TOK:(half + 1) * TOK].astype(np.int32)
        m["pos_t"] = np.ascontiguousarray(
            pos[half * TOK:(half + 1) * TOK].T.astype(np.float32))
        m["enc_t"] = np.ascontiguousarray(enc_out[b].T).astype(BF16)
        m["tmask"] = np.ascontiguousarray(
            tgt_mask[b, 0, half * TOK:(half + 1) * TOK, :])
        m["smask"] = np.ascontiguousarray(src_mask[b, 0, 0, :][None, :])
        m["fcw"] = _tiled(fcw_pad[c * VS:(c + 1) * VS], FT, ET)
        m["fcb"] = np.ascontiguousarray(fcb_pad[c * VS:(c + 1) * VS])
        in_maps.append(m)
    return in_maps


def assemble_output(results):
    """results: list of per-core {'out': [VS, 2048] f32} -> [N, T, V] f32."""
    full = np.empty((N, T, VPAD), np.float32)
    for c in range(NC_CORES):
        oc = np.asarray(results[c]["out"]).reshape(VS, NC_CORES, TOK)
        for r in range(NC_CORES):
            b, half = r // 2, r % 2
            full[b, half * TOK:(half + 1) * TOK, c * VS:(c + 1) * VS] = \
                oc[:, r, :].T
    return np.ascontiguousarray(full[:, :, :V])


def kernel(x, enc_out, src_mask, tgt_mask, params, trace=False):
    if "nc" not in _CACHE:
        _CACHE["nc"] = build_nc()
    nc = _CACHE["nc"]
    in_maps = prepare_in_maps(x, enc_out, src_mask, tgt_mask, params)
    res = run_bass_kernel_spmd(nc, in_maps, list(range(NC_CORES)), trace=False)
    return assemble_output(res.results)


def run_timed(in_maps, nc=None, iters=8):
    """Mirror bass2jax.run_bass_via_pjrt's shard_map path without output
    donation so the staged inputs can be re-executed and timed.
    Returns (results_list, per_iter_wall_seconds)."""
    import time
    import jax
    from jax.sharding import Mesh, PartitionSpec
    from jax.experimental.shard_map import shard_map
    from concourse import bass2jax as b2j
    from concourse import mybir as _mb

    if nc is None:
        if "nc" not in _CACHE:
            _CACHE["nc"] = build_nc()
        nc = _CACHE["nc"]
    b2j.install_neuronx_cc_hook()
    n_cores = NC_CORES
    partition_name = nc.partition_id_tensor.name if nc.partition_id_tensor else None
    in_names, out_names, out_avals = [], [], []
    for alloc in nc.m.functions[0].allocations:
        if not isinstance(alloc, _mb.MemoryLocationSet):
            continue
        name = alloc.memorylocations[0].name
        if alloc.kind == "ExternalInput":
            if name != partition_name:
                in_names.append(name)
        elif alloc.kind == "ExternalOutput":
            out_names.append(name)
            out_avals.append(jax.core.ShapedArray(
                tuple(alloc.tensor_shape), _mb.dt.np(alloc.dtype)))
    n_params = len(in_names)
    all_names = in_names + out_names
    if partition_name is not None:
        all_names.append(partition_name)

    def _body(*args):
        operands = list(args)
        # outputs are fully written by the kernel; pass fresh zeros anyway
        if partition_name is not None:
            operands.append(b2j.partition_id_tensor())
        outs = b2j._bass_exec_p.bind(
            *operands,
            out_avals=tuple(out_avals),
            in_names=tuple(all_names),
            out_names=tuple(out_names),
            lowering_input_output_aliases=(),
            sim_require_finite=True,
            sim_require_nnan=True,
            nc=nc,
        )
        return tuple(outs)

    devices = jax.devices()[:n_cores]
    mesh = Mesh(np.array(devices), ("core",))
    nouts = len(out_names)
    sharded = jax.jit(
        shard_map(_body, mesh=mesh,
                  in_specs=(PartitionSpec("core"),) * (n_params + nouts),
                  out_specs=(PartitionSpec("core"),) * nouts,
                  check_rep=False),
        keep_unused=True)
    concat_in = [np.concatenate([np.asarray(in_maps[c][nm]) for c in range(n_cores)],
                                axis=0) for nm in in_names]
    concat_zeros = [np.zeros((n_cores * a.shape[0], *a.shape[1:]), a.dtype)
                    for a in out_avals]
    from jax.sharding import NamedSharding
    sh = [NamedSharding(mesh, PartitionSpec("core"))] * (n_params + nouts)
    staged = [jax.device_put(a, s) for a, s in zip(concat_in + concat_zeros, sh)]
    out = sharded(*staged)
    jax.block_until_ready(out)
    times = []
    for _ in range(iters):
        t0 = time.perf_counter()
        out = sharded(*staged)
        jax.block_until_ready(out)
        times.append(time.perf_counter() - t0)
    results = [
        {nm: np.asarray(out[i]).reshape(n_cores, *out_avals[i].shape)[c]
         for i, nm in enumerate(out_names)}
        for c in range(n_cores)
    ]
    return results, times


def measure_exec_ns(in_maps, nc=None, reps=(1, 9), iters=6):
    """Estimate on-device NEFF time: run the kernel body k times inside one
    jit (bass_exec is effectful -> serialized, not CSE'd); the wall-time
    slope between rep counts is the per-execution device time, independent
    of the ~90ms axon dispatch floor."""
    import time
    import jax
    from jax.sharding import Mesh, PartitionSpec, NamedSharding
    from jax.experimental.shard_map import shard_map
    from concourse import bass2jax as b2j
    from concourse import mybir as _mb

    if nc is None:
        if "nc" not in _CACHE:
            _CACHE["nc"] = build_nc()
        nc = _CACHE["nc"]
    b2j.install_neuronx_cc_hook()
    n_cores = NC_CORES
    partition_name = nc.partition_id_tensor.name if nc.partition_id_tensor else None
    in_names, out_names, out_avals = [], [], []
    for alloc in nc.m.functions[0].allocations:
        if not isinstance(alloc, _mb.MemoryLocationSet):
            continue
        name = alloc.memorylocations[0].name
        if alloc.kind == "ExternalInput":
            if name != partition_name:
                in_names.append(name)
        elif alloc.kind == "ExternalOutput":
            out_names.append(name)
            out_avals.append(jax.core.ShapedArray(
                tuple(alloc.tensor_shape), _mb.dt.np(alloc.dtype)))
    n_params = len(in_names)
    all_names = in_names + out_names
    if partition_name is not None:
        all_names.append(partition_name)

    def body_k(k):
        def _body(*args):
            outs = None
            for _ in range(k):
                operands = list(args)
                if partition_name is not None:
                    operands.append(b2j.partition_id_tensor())
                outs = b2j._bass_exec_p.bind(
                    *operands,
                    out_avals=tuple(out_avals),
                    in_names=tuple(all_names),
                    out_names=tuple(out_names),
                    lowering_input_output_aliases=(),
                    sim_require_finite=True,
                    sim_require_nnan=True,
                    nc=nc,
                )
            return tuple(outs)
        return _body

    devices = jax.devices()[:n_cores]
    mesh = Mesh(np.array(devices), ("core",))
    nouts = len(out_names)
    concat_in = [np.concatenate([np.asarray(in_maps[c][nm]) for c in range(n_cores)],
                                axis=0) for nm in in_names]
    concat_zeros = [np.zeros((n_cores * a.shape[0], *a.shape[1:]), a.dtype)
                    for a in out_avals]
    sh = [NamedSharding(mesh, PartitionSpec("core"))] * (n_params + nouts)
    staged = [jax.device_put(a, s) for a, s in zip(concat_in + concat_zeros, sh)]

    med = {}
    for k in reps:
        f = jax.jit(
            shard_map(body_k(k), mesh=mesh,
                      in_specs=(PartitionSpec("core"),) * (n_params + nouts),
                      out_specs=(PartitionSpec("core"),) * nouts,
                      check_rep=False),
            keep_unused=True)
        out = f(*staged)
        jax.block_until_ready(out)
        ts = []
        for _ in range(iters):
            t0 = time.perf_counter()
            out = f(*staged)
            jax.block_until_ready(out)
            ts.append(time.perf_counter() - t0)
        ts.sort()
        med[k] = ts[len(ts) // 2]
        print(f"  reps={k}: median wall {med[k]*1e3:.3f} ms "
              f"(all: {[round(t*1e3,1) for t in ts]})")
    k0, k1 = reps[0], reps[-1]
    exec_ns = (med[k1] - med[k0]) / (k1 - k0) * 1e9
    return int(exec_ns)
